# revision 12
# baseline (speedup 1.0000x reference)
"""BlockTucker fusion kernel for 8 Trainium2 NeuronCores — v5.

Reference computation (per batch row b):
    h0 = x0 @ W0 + b0; h1 = x1 @ W1 + b1              # [B, 1600]
    per chunk c (20 chunks of 80):
        z[c,o] = sum_{s,t} h0c[s] Wb[c,o,s,t] h1c[t] + bb[c,o]
        z = signsqrt(z); z /= max(||z||_2, 1e-12)
    out = concat(z) @ Wout + bout                      # [B, 3000]

v5 vs v2/v3 (~1.3ms): HW ablation showed the kernel is bound by the DMA
path; the h0/h1 SBUF replication DMAs alone move ~150MB/body (write +
broadcast-read amplification) at ~260GB/s = ~580us.  v5 eliminates them:

  - Projections are CHUNK-ALIGNED: 20 m-tiles per input, each holding one
    chunk's 80 features in rows 0..79 (weight cols 80..127 zero-padded).
    Costs +~220 matmul passes, but every chunk's h is a partition-aligned
    SBUF tile straight out of the projection.
  - The replicated bilinear feeds (h0 row on 16 partitions / h1 row on 8)
    are built ON THE PE with 0/1 indicator matmuls (lhsT = Lrep slot
    matrix, rhs = the chunk's h tile) and evacuated PSUM->SBUF by the ACT
    engine (~0.43us per [128,512] slice).  No replication DMAs at all.
  - h1 round-trips through DRAM compactly (3.3MB) so x1's SBUF can be
    reused; h0 feeds straight from SBUF.
  - rn (the per-chunk L2 norms) is broadcast to 80 partitions the same
    way (indicator matmul + ACT copy), killing the rn DRAM round trip.
  - replica matmuls for chunk c+1 are emitted interleaved between chunk
    c's bilinear i-groups so the in-order PE queue never stalls on the
    ACT evacuation ping-pong.
"""

import sys

sys.path.insert(0, "/opt/trn_rl_repo")

from contextlib import ExitStack

import numpy as np
import ml_dtypes

import concourse.bass as bass
import concourse.mybir as mybir
import concourse.tile as tile
from concourse import bacc
from concourse.bass_utils import run_bass_kernel_spmd

BF16 = mybir.dt.bfloat16
F32 = mybir.dt.float32
AF = mybir.ActivationFunctionType

B = 8192
D_IN = 2048
MM = 1600
CHUNKS = 20
CS = 80
D_OUT = 3000
N_CORES = 8
BL = B // N_CORES  # 1024 batch rows per core

K_IN = D_IN // 128  # 16 k-tiles for projections
MT_H = 13  # dense z k-tiles (1664 rows) for the out projection
MM_PAD = MT_H * 128  # 1664
KT_BIL = 50  # k-tiles per chunk for bilinear (6400/128)
MT_O = 24  # m-tiles for out (3000 -> 23x128 + 56)
D_OUT_PAD = MT_O * 128  # 3072
NH = BL // 512  # 2 free-dim halves of 512
HALF = 2  # norm-batch granularity (chunks)
NSLOT = 15  # replica slots: 10 h0 (i) + 5 h1 (j)

CFG = dict(
    out_bf16=True,    # outT in bf16 (halves output DMA bytes)
    rnb_pe=True,      # rn broadcast via indicator matmul (False: DMA bcast)
    abl_pt=True,      # ablation: False = reuse 2 memset pt tiles (no DVE)
    abl_bilmm=True,   # ablation: False = skip bilinear matmuls
    abl_repl=True,    # ablation: False = skip replica matmul/evac ops
)


def build_program(reps=1):
    nc = bacc.Bacc("TRN2", target_bir_lowering=False, debug=False)

    x0T = nc.dram_tensor("x0T", [D_IN, BL], BF16, kind="ExternalInput").ap()
    x1T = nc.dram_tensor("x1T", [D_IN, BL], BF16, kind="ExternalInput").ap()
    w0 = nc.dram_tensor(
        "w0c", [128, CHUNKS, K_IN * 128], BF16, kind="ExternalInput"
    ).ap()
    w1 = nc.dram_tensor(
        "w1c", [128, CHUNKS, K_IN * 128], BF16, kind="ExternalInput"
    ).ap()
    wbp = nc.dram_tensor(
        "wbpt", [CHUNKS, 128, KT_BIL, CS], BF16, kind="ExternalInput"
    ).ap()
    wout = nc.dram_tensor("wout", [128, MT_O, MT_H * 128], BF16, kind="ExternalInput").ap()
    b0c = nc.dram_tensor("b0cc", [128, CHUNKS], F32, kind="ExternalInput").ap()
    b1c = nc.dram_tensor("b1cc", [128, CHUNKS], F32, kind="ExternalInput").ap()
    bbT = nc.dram_tensor("bbT", [CS, CHUNKS], F32, kind="ExternalInput").ap()
    boutc = nc.dram_tensor("boutc", [128, MT_O], F32, kind="ExternalInput").ap()
    lrep = nc.dram_tensor("lrep", [CS, NSLOT, 128], BF16, kind="ExternalInput").ap()
    lrn = nc.dram_tensor("lrn", [CHUNKS, CHUNKS, 128], BF16, kind="ExternalInput").ap()
    outT = nc.dram_tensor(
        "outT", [D_OUT, BL], BF16 if CFG["out_bf16"] else F32,
        kind="ExternalOutput"
    ).ap()
    h1_dram = nc.dram_tensor("h1_dram", [MM, BL], BF16).ap()
    z_dram = nc.dram_tensor("z_dram", [MM_PAD, BL], BF16).ap()
    rn_dram = nc.dram_tensor("rn_dram", [CHUNKS, BL], BF16).ap()

    with tile.TileContext(nc) as tc:
        for _ in range(reps):
            _emit(
                tc, nc, x0T, x1T, w0, w1, wbp, wout, b0c, b1c, bbT, boutc,
                lrep, lrn, outT, h1_dram, z_dram, rn_dram,
            )
    nc.compile()
    return nc


def _emit(
    tc, nc, x0T, x1T, w0, w1, wbp, wout, b0c, b1c, bbT, boutc, lrep, lrn,
    outT, h1_dram, z_dram, rn_dram,
):
    ctx = ExitStack()
    with ctx:
        singles = ctx.enter_context(tc.tile_pool(name="singles", bufs=1))
        mm_psum = ctx.enter_context(tc.tile_pool(name="mm_psum", bufs=2, space="PSUM"))
        rp_psum = ctx.enter_context(tc.tile_pool(name="rp_psum", bufs=2, space="PSUM"))
        zpsum_pool = ctx.enter_context(tc.tile_pool(name="zpsum", bufs=1, space="PSUM"))
        nsq_psum = ctx.enter_context(tc.tile_pool(name="nsq_psum", bufs=1, space="PSUM"))

        # chunk-lifetime pools first (pool allocator is a stack; projection
        # pools go innermost so they can be freed for the out-proj pools)
        mid = ctx.enter_context(ExitStack())
        rep_pool = mid.enter_context(tc.tile_pool(name="rep", bufs=4))
        dup_pool = mid.enter_context(tc.tile_pool(name="dup", bufs=2))
        wb_pool = mid.enter_context(tc.tile_pool(name="wbpool", bufs=2))
        p_pool = mid.enter_context(tc.tile_pool(name="ppool", bufs=2))
        post_pool = mid.enter_context(tc.tile_pool(name="post", bufs=2))
        zs_pool = mid.enter_context(tc.tile_pool(name="zs", bufs=2))
        nsq1_pool = mid.enter_context(tc.tile_pool(name="nsq1", bufs=1))
        rnb_pool = mid.enter_context(tc.tile_pool(name="rnb", bufs=2))
        zn_pool = mid.enter_context(tc.tile_pool(name="zn", bufs=2))
        h1c_pool = mid.enter_context(tc.tile_pool(name="h1c", bufs=2))
        h0e_pool = mid.enter_context(tc.tile_pool(name="h0e", bufs=2))
        proj_es = ExitStack()
        xpool = proj_es.enter_context(tc.tile_pool(name="xpool", bufs=1))
        wproj = proj_es.enter_context(tc.tile_pool(name="wproj", bufs=2))
        h1e_pool = proj_es.enter_context(tc.tile_pool(name="h1e", bufs=2))

        # constants / biases
        b0s = singles.tile([128, CHUNKS], F32)
        nc.sync.dma_start(out=b0s, in_=b0c)
        b1s = singles.tile([128, CHUNKS], F32)
        nc.sync.dma_start(out=b1s, in_=b1c)
        bbs = singles.tile([CS, CHUNKS], F32)
        nc.sync.dma_start(out=bbs, in_=bbT)
        bouts = singles.tile([128, MT_O], F32)
        nc.sync.dma_start(out=bouts, in_=boutc)
        lreps = singles.tile([CS, NSLOT, 128], BF16)
        nc.sync.dma_start(
            out=lreps.rearrange("p s m -> p (s m)"),
            in_=lrep.rearrange("p s m -> p (s m)"),
        )
        lrns = singles.tile([CHUNKS, CHUNKS, 128], BF16)
        nc.sync.dma_start(
            out=lrns.rearrange("p s m -> p (s m)"),
            in_=lrn.rearrange("p s m -> p (s m)"),
        )
        ones80 = singles.tile([CS, 1], BF16)
        nc.vector.memset(ones80, 1.0)
        # zero the z_dram pad rows once (wout pad rows are zero, but 0*NaN)
        zpad = singles.tile([64, BL], BF16)
        nc.vector.memset(zpad, 0.0)
        nc.sync.dma_start(out=z_dram[MM:MM_PAD, :], in_=zpad)

        nsq_big = singles.tile([CHUNKS, BL], F32)
        nc.vector.memset(nsq_big, 1.0)
        rn_big = singles.tile([CHUNKS, BL], F32)
        rn_bf16 = singles.tile([CHUNKS, BL], BF16, tag="rn_bf16")

        def load_x(xT):
            xr = xT.rearrange("(kt p) b -> p kt b", p=128)
            xt = xpool.tile([128, K_IN, BL], BF16, tag="x")
            for halfk in range(2):
                nc.sync.dma_start(
                    out=xt[:, 8 * halfk : 8 * halfk + 8, :],
                    in_=xr[:, 8 * halfk : 8 * halfk + 8, :],
                )
            return xt

        def emit_proj(c, xt, wdram, bias_s, ev_pool, tag):
            """One chunk-aligned projection m-tile -> [CS, BL] SBUF tile."""
            wt = wproj.tile([128, K_IN, 128], BF16, tag="wt")
            nc.sync.dma_start(
                out=wt.rearrange("p k m -> p (k m)"), in_=wdram[:, c, :]
            )
            hev = ev_pool.tile([CS, BL], BF16, tag=tag)
            for h in range(NH):
                ps = mm_psum.tile([128, 512], F32, tag="mmps")
                for kt in range(K_IN):
                    nc.tensor.matmul(
                        out=ps,
                        lhsT=wt[:, kt, :],
                        rhs=xt[:, kt, h * 512 : (h + 1) * 512],
                        start=(kt == 0),
                        stop=(kt == K_IN - 1),
                    )
                nc.scalar.activation(
                    out=hev[:, h * 512 : (h + 1) * 512],
                    in_=ps[:CS, :],
                    func=AF.Identity,
                    bias=bias_s[:CS, c : c + 1],
                    scale=1.0,
                )
            return hev

        # ---- phase A: h1 projection (serial), compact write to DRAM ----
        xt1 = load_x(x1T)
        for c in range(CHUNKS):
            hev1 = emit_proj(c, xt1, w1, b1s, h1e_pool, "h1ev")
            nc.sync.dma_start(out=h1_dram[CS * c : CS * c + CS, :], in_=hev1)

        # ---- phase B: h0 projection interleaved with bilinear chunks ----
        xt0 = load_x(x0T)

        def make_replicas(c, hev0):
            """Replica tiles for chunk c + the PE/ACT ops that fill them
            (returned as closures, drained inside the previous chunk's
            compute so the PE queue never stalls on PSUM evacuation)."""
            h1cmp = h1c_pool.tile([CS, BL], BF16, tag="h1c")
            nc.sync.dma_start(out=h1cmp, in_=h1_dram[CS * c : CS * c + CS, :])
            wbt = wb_pool.tile([128, KT_BIL, 128], BF16, tag="wbt")
            if c < 2 or CFG.get("wb_memset_all"):
                # zero FWL pad cols once per pool buffer; later chunks reuse
                # the bytes and only rewrite [:, :, :CS]
                nc.vector.memset(wbt, 0.0)
            nc.sync.dma_start(out=wbt[:, :, :CS], in_=wbp[c])
            if not CFG["abl_repl"]:
                if "repl" not in _abl:
                    h0rep = []
                    for _ih in range(2):
                        hrt = rep_pool.tile([128, 5, BL], BF16, tag="h0rep")
                        nc.gpsimd.memset(hrt, 0.5)
                        h0rep.append(hrt)
                    h1dup = dup_pool.tile([128, 5, BL], BF16, tag="h1dup")
                    nc.gpsimd.memset(h1dup, 0.5)
                    _abl["repl"] = (h0rep, h1dup)
                h0rep, h1dup = _abl["repl"]
                return h0rep, h1dup, wbt, []
            h0rep = []
            for _ih in range(2):
                hrt = rep_pool.tile([128, 5, BL], BF16, tag="h0rep")
                h0rep.append(hrt)
            h1dup = dup_pool.tile([128, 5, BL], BF16, tag="h1dup")

            ops = []
            for h in range(NH):
                cols = slice(h * 512, (h + 1) * 512)
                for slot in range(NSLOT):
                    def op(h=h, cols=cols, slot=slot):
                        ps = rp_psum.tile([128, 512], F32, tag="rps")
                        if slot < 10:
                            src, dst = hev0, h0rep[slot // 5][:, slot % 5, cols]
                        else:
                            src, dst = h1cmp, h1dup[:, slot - 10, cols]
                        nc.tensor.matmul(
                            out=ps,
                            lhsT=lreps[:, slot, :],
                            rhs=src[:, cols],
                            start=True,
                            stop=True,
                            skip_group_check=True,
                        )
                        nc.scalar.copy(dst, ps)
                    ops.append(op)
            return h0rep, h1dup, wbt, ops

        zs_tiles = {}
        _abl = {}

        def emit_compute(c, feeds, bg):
            """Bilinear + post + norms for chunk c, draining bg ops (the
            next chunk's replica builds) between i-groups."""
            h0rep, h1dup, wbt = feeds
            bgi = 0

            av = post_pool.tile([CS, BL], BF16, tag="av")
            gv = post_pool.tile([CS, BL], BF16, tag="gv")
            nps = nsq_psum.tile([1, BL], F32, tag="nps")
            # full-BL pt tiles + one 2-bank zps: halves the DVE op count and
            # the DVE<->PE sync hops vs per-half tiles.  h outer within i so
            # the PSUM bank switches every 5 matmuls, not every matmul.
            zps = zpsum_pool.tile([128, BL], F32, tag="zps")
            for i in range(10):
                pt = p_pool.tile([128, 5, BL], BF16, tag="pt")
                nc.vector.tensor_mul(
                    pt,
                    h0rep[i // 5][:, i % 5, :]
                    .unsqueeze(1)
                    .broadcast_to([128, 5, BL]),
                    h1dup,
                )
                for h in range(NH):
                    cols = slice(h * 512, (h + 1) * 512)
                    for j in range(5):
                        kt = 5 * i + j
                        nc.tensor.matmul(
                            out=zps[:, cols],
                            lhsT=wbt[:, kt, :],
                            rhs=pt[:, j, cols],
                            start=(kt == 0),
                            stop=(kt == KT_BIL - 1),
                        )
                # drain up to 3 pending replica builds for chunk c+1
                for _ in range(3):
                    if bgi < len(bg):
                        bg[bgi]()
                        bgi += 1
            nc.scalar.activation(
                out=av, in_=zps[:CS, :], func=AF.Abs,
                bias=bbs[:, c : c + 1], scale=1.0,
            )
            nc.scalar.activation(
                out=gv, in_=zps[:CS, :], func=AF.Sign,
                bias=bbs[:, c : c + 1], scale=1.0,
            )
            for h in range(NH):
                cols = slice(h * 512, (h + 1) * 512)
                nc.tensor.matmul(
                    out=nps[:, cols],
                    lhsT=ones80,
                    rhs=av[:, cols],
                    start=True,
                    stop=True,
                    skip_group_check=True,
                )
            while bgi < len(bg):
                bg[bgi]()
                bgi += 1
            sv = post_pool.tile([CS, BL], BF16, tag="sv")
            nc.scalar.activation(out=sv, in_=av, func=AF.Sqrt)
            zst = zs_pool.tile([CS, BL], BF16, tag="zst")
            nc.gpsimd.tensor_mul(zst, sv, gv)
            zs_tiles[c] = zst
            nsq1 = nsq1_pool.tile([1, BL], F32, tag="nsq1")
            nc.scalar.copy(nsq1, nps)
            nc.sync.dma_start(out=nsq_big[c : c + 1, :], in_=nsq1)

            if c % HALF == HALF - 1:
                lo, hi = c - HALF + 1, c + 1
                # ACT/DVE need start-partition 0: recompute the whole strip
                nc.scalar.activation(out=rn_big, in_=nsq_big, func=AF.Sqrt)
                nc.vector.tensor_scalar_max(rn_big, rn_big, 1e-12)
                with nc.allow_low_precision(reason="rn applied to bf16 z"):
                    nc.vector.reciprocal(rn_bf16, rn_big)
                if not CFG["rnb_pe"]:
                    nc.sync.dma_start(
                        out=rn_dram[lo:hi, :], in_=rn_bf16[lo:hi]
                    )
                for cc in range(lo, hi):
                    # rn[cc] broadcast to 80 partitions via indicator matmul
                    rnb = rnb_pool.tile([CS, BL], BF16, tag="rnb")
                    if CFG["rnb_pe"]:
                        for h in range(NH):
                            cols = slice(h * 512, (h + 1) * 512)
                            ps = rp_psum.tile([128, 512], F32, tag="rps")
                            nc.tensor.matmul(
                                out=ps,
                                lhsT=lrns[:, cc, :],
                                rhs=rn_bf16[:, cols],
                                start=True,
                                stop=True,
                                skip_group_check=True,
                            )
                            nc.scalar.copy(rnb[:, cols], ps[:CS, :])
                    else:
                        nc.sync.dma_start(
                            out=rnb,
                            in_=rn_dram[cc : cc + 1, :].partition_broadcast(CS),
                        )
                    zn = zn_pool.tile([CS, BL], BF16, tag="zn")
                    nc.gpsimd.tensor_mul(zn, zs_tiles.pop(cc), rnb)
                    nc.sync.dma_start(
                        out=z_dram[CS * cc : CS * cc + CS, :], in_=zn
                    )

        feeds = {}
        prev = None
        for c in range(CHUNKS):
            hev0 = emit_proj(c, xt0, w0, b0s, h0e_pool, "h0ev")
            h0rep, h1dup, wbt, ops = make_replicas(c, hev0)
            feeds[c] = (h0rep, h1dup, wbt)
            if prev is None:
                for op in ops:
                    op()
            else:
                emit_compute(prev, feeds.pop(prev), ops)
            prev = c
        proj_es.close()
        out_pools = {
            "zk": ctx.enter_context(tc.tile_pool(name="zk", bufs=1)),
            "wo": ctx.enter_context(tc.tile_pool(name="wo", bufs=2)),
            "o": ctx.enter_context(tc.tile_pool(name="opool", bufs=2)),
        }
        emit_compute(prev, feeds.pop(prev), [])

        # ---- out projection: out^T = Wout^T @ z + bout ----
        ZKG = (5, 5, 3)  # k-tile groups for the z reload
        zk_pool = out_pools["zk"]
        wo_pool = out_pools["wo"]
        o_pool = out_pools["o"]
        zk = []
        kt0 = 0
        for gi, gn in enumerate(ZKG):
            zt = zk_pool.tile([128, gn, BL], BF16, tag=f"zk{gi}")
            nc.sync.dma_start(
                out=zt,
                in_=z_dram[kt0 * 128 : (kt0 + gn) * 128, :].rearrange(
                    "(kt p) b -> p kt b", p=128
                ),
            )
            for k in range(gn):
                zk.append((zt, k))
            kt0 += gn
        for mt in range(MT_O):
            m0 = mt * 128
            mw = min(128, D_OUT - m0)
            wot = wo_pool.tile([128, MT_H, 128], BF16, tag="wot")
            nc.sync.dma_start(
                out=wot.rearrange("p k m -> p (k m)"), in_=wout[:, mt, :]
            )
            ot = o_pool.tile(
                [128, BL], BF16 if CFG["out_bf16"] else F32, tag="ot"
            )
            for h in range(NH):
                ps = mm_psum.tile([128, 512], F32, tag="mmps")
                for kt in range(MT_H):
                    zt, k = zk[kt]
                    nc.tensor.matmul(
                        out=ps,
                        lhsT=wot[:, kt, :],
                        rhs=zt[:, k, h * 512 : (h + 1) * 512],
                        start=(kt == 0),
                        stop=(kt == MT_H - 1),
                    )
                nc.scalar.activation(
                    out=ot[:mw, h * 512 : (h + 1) * 512],
                    in_=ps[:mw, :],
                    func=AF.Identity,
                    bias=bouts[:mw, mt : mt + 1],
                    scale=1.0,
                )
            nc.sync.dma_start(out=outT[m0 : m0 + mw, :], in_=ot[:mw, :])


_PROGRAM = None


def _get_program():
    global _PROGRAM
    if _PROGRAM is None:
        _PROGRAM = build_program()
    return _PROGRAM


def prep_weights(W0, b0, W1, b1, Wb, bb, Wout, bout):
    bf = ml_dtypes.bfloat16

    def pack_proj_chunk(W):
        # [K, 1600] -> chunk-aligned [p, c, kt*128+m]: m-tile c holds the
        # chunk's 80 features in cols 0..79, cols 80..127 zero
        K = W.shape[0]
        Wp = np.zeros((K, CHUNKS * 128), np.float32)
        for c in range(CHUNKS):
            Wp[:, 128 * c : 128 * c + CS] = W[:, CS * c : CS * c + CS]
        kt_n = K // 128
        return np.ascontiguousarray(
            Wp.reshape(kt_n, 128, CHUNKS, 128).transpose(1, 2, 0, 3)
            .reshape(128, CHUNKS, kt_n * 128)
        ).astype(bf)

    def pack_bias_chunk(b):
        bp = np.zeros((128, CHUNKS), np.float32)
        for c in range(CHUNKS):
            bp[:CS, c] = b[CS * c : CS * c + CS]
        return np.ascontiguousarray(bp)

    w0 = pack_proj_chunk(np.asarray(W0, np.float32))
    w1 = pack_proj_chunk(np.asarray(W1, np.float32))
    b0cc = pack_bias_chunk(np.asarray(b0, np.float32))
    b1cc = pack_bias_chunk(np.asarray(b1, np.float32))
    # wbp[c, p, 5i+j, o] = Wb[c, o, 10*(p//16) + i, 5*(p%16) + j]
    p = np.arange(128)
    wbpt = np.empty((CHUNKS, 128, KT_BIL, CS), dtype=bf)
    for i in range(10):
        s_idx = 10 * (p // 16) + i
        for j in range(5):
            t_idx = 5 * (p % 16) + j
            wbpt[:, :, 5 * i + j, :] = Wb[:, :, s_idx, t_idx].transpose(0, 2, 1)
    # replica indicator matrices: slot<10: h0 row 10*(p//16)+i on partition
    # p; slot>=10: h1 row 5*(p%16)+j
    lrep = np.zeros((CS, NSLOT, 128), np.float32)
    for i in range(10):
        lrep[10 * (p // 16) + i, i, p] = 1.0
    for j in range(5):
        lrep[5 * (p % 16) + j, 10 + j, p] = 1.0
    lrepb = lrep.astype(bf)
    # rn broadcast indicators: out[p] = rn[cc]
    lrn = np.zeros((CHUNKS, CHUNKS, 128), np.float32)
    for cc in range(CHUNKS):
        lrn[cc, cc, :] = 1.0
    lrnb = lrn.astype(bf)
    Woutp = np.zeros((MM_PAD, D_OUT_PAD), np.float32)
    Woutp[:MM, :D_OUT] = Wout
    woutp = np.ascontiguousarray(
        Woutp.reshape(MT_H, 128, MT_O, 128).transpose(1, 2, 0, 3)
        .reshape(128, MT_O, MT_H * 128)
    ).astype(bf)
    bbT = np.ascontiguousarray(np.asarray(bb, np.float32).T)
    boutp = np.zeros(D_OUT_PAD, np.float32)
    boutp[:D_OUT] = bout
    boutc = np.ascontiguousarray(boutp.reshape(MT_O, 128).T)
    return dict(
        w0c=w0, w1c=w1, wbpt=wbpt, wout=woutp, b0cc=b0cc, b1cc=b1cc,
        bbT=bbT, boutc=boutc, lrep=lrepb, lrn=lrnb,
    )


def make_in_maps(x0, x1, weights):
    bf = ml_dtypes.bfloat16
    x0T = np.ascontiguousarray(np.asarray(x0, np.float32).T).astype(bf)
    x1T = np.ascontiguousarray(np.asarray(x1, np.float32).T).astype(bf)
    in_maps = []
    for r in range(N_CORES):
        sl = slice(r * BL, (r + 1) * BL)
        m = dict(weights)
        m["x0T"] = np.ascontiguousarray(x0T[:, sl])
        m["x1T"] = np.ascontiguousarray(x1T[:, sl])
        in_maps.append(m)
    return in_maps


def run(x0, x1, weights, **kwargs):
    nc = _get_program()
    in_maps = make_in_maps(x0, x1, weights)
    res = run_bass_kernel_spmd(nc, in_maps, core_ids=list(range(N_CORES)), **kwargs)
    out = np.empty((B, D_OUT), np.float32)
    for r in range(N_CORES):
        out[r * BL : (r + 1) * BL, :] = np.asarray(
            res.results[r]["outT"], np.float32
        ).T
    return out, res


def kernel(x0, x1, W0, b0, W1, b1, Wb, bb, Wout, bout):
    weights = prep_weights(W0, b0, W1, b1, Wb, bb, Wout, bout)
    out, _ = run(x0, x1, weights)
    return out


# ---- timed runner (no NTFF hook in this container: wall-clock the PJRT
# executable with device-resident inputs, minus dispatch overhead) ----

def _make_sharded_callable(nc, in_maps):
    import jax
    import numpy as _np
    from jax.sharding import Mesh, PartitionSpec, NamedSharding
    from jax.experimental.shard_map import shard_map
    from concourse import bass2jax as b2j
    from concourse import mybir as _mybir

    b2j.install_neuronx_cc_hook()
    n_cores = len(in_maps)
    partition_name = nc.partition_id_tensor.name if nc.partition_id_tensor else None
    in_names, out_names, out_avals, zero_outs = [], [], [], []
    for alloc in nc.m.functions[0].allocations:
        if not isinstance(alloc, _mybir.MemoryLocationSet):
            continue
        name = alloc.memorylocations[0].name
        if alloc.kind == "ExternalInput":
            if name != partition_name:
                in_names.append(name)
        elif alloc.kind == "ExternalOutput":
            shape = tuple(alloc.tensor_shape)
            dtype = _mybir.dt.np(alloc.dtype)
            out_names.append(name)
            out_avals.append(jax.core.ShapedArray(shape, dtype))
            zero_outs.append(_np.zeros(shape, dtype))
    n_params = len(in_names)
    in_names_all = list(in_names) + list(out_names)
    if partition_name is not None:
        in_names_all.append(partition_name)

    def _body(*args):
        operands = list(args)
        if partition_name is not None:
            operands.append(b2j.partition_id_tensor())
        outs = b2j._bass_exec_p.bind(
            *operands,
            out_avals=tuple(out_avals),
            in_names=tuple(in_names_all),
            out_names=tuple(out_names),
            lowering_input_output_aliases=(),
            sim_require_finite=True,
            sim_require_nnan=True,
            nc=nc,
        )
        return tuple(outs)

    devices = jax.devices()[:n_cores]
    mesh = Mesh(_np.asarray(devices), ("core",))
    spec = PartitionSpec("core")
    in_specs = (spec,) * (n_params + len(out_names))
    out_specs = (spec,) * len(out_names)
    n_outs = len(out_names)
    donate = tuple(range(n_params, n_params + n_outs))
    sharded = jax.jit(
        shard_map(_body, mesh=mesh, in_specs=in_specs, out_specs=out_specs,
                  check_rep=False),
        keep_unused=True,
        donate_argnums=donate,
    )
    sh = NamedSharding(mesh, spec)
    concat_in = [
        jax.device_put(
            _np.concatenate([_np.asarray(in_maps[c][n]) for c in range(n_cores)], 0), sh
        )
        for n in in_names
    ]
    state = {"outs": None}

    def _fresh_zeros():
        return [
            jax.device_put(_np.zeros((n_cores * z.shape[0], *z.shape[1:]), z.dtype), sh)
            for z in zero_outs
        ]

    def call():
        seeds = state["outs"] if state["outs"] is not None else _fresh_zeros()
        outs = sharded(*concat_in, *seeds)
        state["outs"] = list(outs)
        return outs
    return call, out_names, out_avals


# revision 13
# speedup vs baseline: 1.0460x; 1.0460x over previous
"""BlockTucker fusion kernel for 8 Trainium2 NeuronCores — v5.

Reference computation (per batch row b):
    h0 = x0 @ W0 + b0; h1 = x1 @ W1 + b1              # [B, 1600]
    per chunk c (20 chunks of 80):
        z[c,o] = sum_{s,t} h0c[s] Wb[c,o,s,t] h1c[t] + bb[c,o]
        z = signsqrt(z); z /= max(||z||_2, 1e-12)
    out = concat(z) @ Wout + bout                      # [B, 3000]

v5 vs v2/v3 (~1.3ms): HW ablation showed the kernel is bound by the DMA
path; the h0/h1 SBUF replication DMAs alone move ~150MB/body (write +
broadcast-read amplification) at ~260GB/s = ~580us.  v5 eliminates them:

  - Projections are CHUNK-ALIGNED: 20 m-tiles per input, each holding one
    chunk's 80 features in rows 0..79 (weight cols 80..127 zero-padded).
    Costs +~220 matmul passes, but every chunk's h is a partition-aligned
    SBUF tile straight out of the projection.
  - The replicated bilinear feeds (h0 row on 16 partitions / h1 row on 8)
    are built ON THE PE with 0/1 indicator matmuls (lhsT = Lrep slot
    matrix, rhs = the chunk's h tile) and evacuated PSUM->SBUF by the ACT
    engine (~0.43us per [128,512] slice).  No replication DMAs at all.
  - h1 round-trips through DRAM compactly (3.3MB) so x1's SBUF can be
    reused; h0 feeds straight from SBUF.
  - rn (the per-chunk L2 norms) is broadcast to 80 partitions the same
    way (indicator matmul + ACT copy), killing the rn DRAM round trip.
  - replica matmuls for chunk c+1 are emitted interleaved between chunk
    c's bilinear i-groups so the in-order PE queue never stalls on the
    ACT evacuation ping-pong.
"""

import sys

sys.path.insert(0, "/opt/trn_rl_repo")

from contextlib import ExitStack

import numpy as np
import ml_dtypes

import concourse.bass as bass
import concourse.mybir as mybir
import concourse.tile as tile
from concourse import bacc
from concourse.bass_utils import run_bass_kernel_spmd

BF16 = mybir.dt.bfloat16
F32 = mybir.dt.float32
AF = mybir.ActivationFunctionType

B = 8192
D_IN = 2048
MM = 1600
CHUNKS = 20
CS = 80
D_OUT = 3000
N_CORES = 8
BL = B // N_CORES  # 1024 batch rows per core

K_IN = D_IN // 128  # 16 k-tiles for projections
MT_H = 13  # dense z k-tiles (1664 rows) for the out projection
MM_PAD = MT_H * 128  # 1664
KT_BIL = 50  # k-tiles per chunk for bilinear (6400/128)
MT_O = 24  # m-tiles for out (3000 -> 23x128 + 56)
D_OUT_PAD = MT_O * 128  # 3072
NH = BL // 512  # 2 free-dim halves of 512
HALF = 2  # norm-batch granularity (chunks)
NSLOT = 15  # replica slots: 10 h0 (i) + 5 h1 (j)

CFG = dict(
    out_bf16=True,    # outT in bf16 (halves output DMA bytes)
    rnb_pe=True,      # rn broadcast via indicator matmul (False: DMA bcast)
    abl_pt=True,      # ablation: False = reuse 2 memset pt tiles (no DVE)
    abl_bilmm=True,   # ablation: False = skip bilinear matmuls
    abl_repl=True,    # ablation: False = skip replica matmul/evac ops
)


def build_program(reps=1):
    nc = bacc.Bacc("TRN2", target_bir_lowering=False, debug=False)

    x0T = nc.dram_tensor("x0T", [D_IN, BL], BF16, kind="ExternalInput").ap()
    x1T = nc.dram_tensor("x1T", [D_IN, BL], BF16, kind="ExternalInput").ap()
    w0 = nc.dram_tensor(
        "w0c", [128, CHUNKS, K_IN * 128], BF16, kind="ExternalInput"
    ).ap()
    w1 = nc.dram_tensor(
        "w1c", [128, CHUNKS, K_IN * 128], BF16, kind="ExternalInput"
    ).ap()
    wbp = nc.dram_tensor(
        "wbpt", [CHUNKS, 128, KT_BIL, CS], BF16, kind="ExternalInput"
    ).ap()
    wout = nc.dram_tensor("wout", [128, MT_O, MT_H * 128], BF16, kind="ExternalInput").ap()
    b0c = nc.dram_tensor("b0cc", [128, CHUNKS], F32, kind="ExternalInput").ap()
    b1c = nc.dram_tensor("b1cc", [128, CHUNKS], F32, kind="ExternalInput").ap()
    bbT = nc.dram_tensor("bbT", [CS, CHUNKS], F32, kind="ExternalInput").ap()
    boutc = nc.dram_tensor("boutc", [128, MT_O], F32, kind="ExternalInput").ap()
    lrep = nc.dram_tensor("lrep", [CS, NSLOT, 128], BF16, kind="ExternalInput").ap()
    lrn = nc.dram_tensor("lrn", [CHUNKS, CHUNKS, 128], BF16, kind="ExternalInput").ap()
    outT = nc.dram_tensor(
        "outT", [D_OUT, BL], BF16 if CFG["out_bf16"] else F32,
        kind="ExternalOutput"
    ).ap()
    h1_dram = nc.dram_tensor("h1_dram", [MM, BL], BF16).ap()
    z_dram = nc.dram_tensor("z_dram", [MM_PAD, BL], BF16).ap()
    rn_dram = nc.dram_tensor("rn_dram", [CHUNKS, BL], BF16).ap()

    with tile.TileContext(nc) as tc:
        for _ in range(reps):
            _emit(
                tc, nc, x0T, x1T, w0, w1, wbp, wout, b0c, b1c, bbT, boutc,
                lrep, lrn, outT, h1_dram, z_dram, rn_dram,
            )
    nc.compile()
    return nc


def _emit(
    tc, nc, x0T, x1T, w0, w1, wbp, wout, b0c, b1c, bbT, boutc, lrep, lrn,
    outT, h1_dram, z_dram, rn_dram,
):
    ctx = ExitStack()
    with ctx:
        singles = ctx.enter_context(tc.tile_pool(name="singles", bufs=1))
        mm_psum = ctx.enter_context(tc.tile_pool(name="mm_psum", bufs=2, space="PSUM"))
        rp_psum = ctx.enter_context(tc.tile_pool(name="rp_psum", bufs=2, space="PSUM"))
        zpsum_pool = ctx.enter_context(tc.tile_pool(name="zpsum", bufs=2, space="PSUM"))
        nsq_psum = ctx.enter_context(tc.tile_pool(name="nsq_psum", bufs=1, space="PSUM"))

        # chunk-lifetime pools first (pool allocator is a stack; projection
        # pools go innermost so they can be freed for the out-proj pools)
        mid = ctx.enter_context(ExitStack())
        rep_pool = mid.enter_context(tc.tile_pool(name="rep", bufs=4))
        dup_pool = mid.enter_context(tc.tile_pool(name="dup", bufs=2))
        wb_pool = mid.enter_context(tc.tile_pool(name="wbpool", bufs=2))
        p_pool = mid.enter_context(tc.tile_pool(name="ppool", bufs=3))
        post_pool = mid.enter_context(tc.tile_pool(name="post", bufs=2))
        zs_pool = mid.enter_context(tc.tile_pool(name="zs", bufs=3))
        nsq1_pool = mid.enter_context(tc.tile_pool(name="nsq1", bufs=2))
        rnb_pool = mid.enter_context(tc.tile_pool(name="rnb", bufs=2))
        zn_pool = mid.enter_context(tc.tile_pool(name="zn", bufs=2))
        h1c_pool = mid.enter_context(tc.tile_pool(name="h1c", bufs=2))
        h0e_pool = mid.enter_context(tc.tile_pool(name="h0e", bufs=2))
        proj_es = ExitStack()
        xpool = proj_es.enter_context(tc.tile_pool(name="xpool", bufs=1))
        wproj = proj_es.enter_context(tc.tile_pool(name="wproj", bufs=2))
        h1e_pool = proj_es.enter_context(tc.tile_pool(name="h1e", bufs=2))

        # constants / biases
        b0s = singles.tile([128, CHUNKS], F32)
        nc.sync.dma_start(out=b0s, in_=b0c)
        b1s = singles.tile([128, CHUNKS], F32)
        nc.sync.dma_start(out=b1s, in_=b1c)
        bbs = singles.tile([CS, CHUNKS], F32)
        nc.sync.dma_start(out=bbs, in_=bbT)
        bouts = singles.tile([128, MT_O], F32)
        nc.sync.dma_start(out=bouts, in_=boutc)
        lreps = singles.tile([CS, NSLOT, 128], BF16)
        nc.sync.dma_start(
            out=lreps.rearrange("p s m -> p (s m)"),
            in_=lrep.rearrange("p s m -> p (s m)"),
        )
        lrns = singles.tile([CHUNKS, CHUNKS, 128], BF16)
        nc.sync.dma_start(
            out=lrns.rearrange("p s m -> p (s m)"),
            in_=lrn.rearrange("p s m -> p (s m)"),
        )
        ones80 = singles.tile([CS, 1], BF16)
        nc.vector.memset(ones80, 1.0)
        # zero the z_dram pad rows once (wout pad rows are zero, but 0*NaN)
        zpad = singles.tile([64, BL], BF16)
        nc.vector.memset(zpad, 0.0)
        nc.sync.dma_start(out=z_dram[MM:MM_PAD, :], in_=zpad)

        nsq_big = singles.tile([CHUNKS, BL], F32)
        nc.vector.memset(nsq_big, 1.0)
        rn_big = singles.tile([CHUNKS, BL], F32)
        rn_bf16 = singles.tile([CHUNKS, BL], BF16, tag="rn_bf16")

        def load_x(xT):
            xr = xT.rearrange("(kt p) b -> p kt b", p=128)
            xt = xpool.tile([128, K_IN, BL], BF16, tag="x")
            for halfk in range(2):
                nc.sync.dma_start(
                    out=xt[:, 8 * halfk : 8 * halfk + 8, :],
                    in_=xr[:, 8 * halfk : 8 * halfk + 8, :],
                )
            return xt

        def emit_proj(c, xt, wdram, bias_s, ev_pool, tag):
            """One chunk-aligned projection m-tile -> [CS, BL] SBUF tile."""
            wt = wproj.tile([128, K_IN, 128], BF16, tag="wt")
            nc.sync.dma_start(
                out=wt.rearrange("p k m -> p (k m)"), in_=wdram[:, c, :]
            )
            hev = ev_pool.tile([CS, BL], BF16, tag=tag)
            for h in range(NH):
                ps = mm_psum.tile([128, 512], F32, tag="mmps")
                for kt in range(K_IN):
                    nc.tensor.matmul(
                        out=ps,
                        lhsT=wt[:, kt, :],
                        rhs=xt[:, kt, h * 512 : (h + 1) * 512],
                        start=(kt == 0),
                        stop=(kt == K_IN - 1),
                    )
                nc.scalar.activation(
                    out=hev[:, h * 512 : (h + 1) * 512],
                    in_=ps[:CS, :],
                    func=AF.Identity,
                    bias=bias_s[:CS, c : c + 1],
                    scale=1.0,
                )
            return hev

        # ---- phase A: h1 projection (serial), compact write to DRAM ----
        xt1 = load_x(x1T)
        for c in range(CHUNKS):
            hev1 = emit_proj(c, xt1, w1, b1s, h1e_pool, "h1ev")
            nc.sync.dma_start(out=h1_dram[CS * c : CS * c + CS, :], in_=hev1)

        # ---- phase B: h0 projection interleaved with bilinear chunks ----
        xt0 = load_x(x0T)

        def make_replicas(c, hev0):
            """Replica tiles for chunk c + the PE/ACT ops that fill them
            (returned as closures, drained inside the previous chunk's
            compute so the PE queue never stalls on PSUM evacuation)."""
            h1cmp = h1c_pool.tile([CS, BL], BF16, tag="h1c")
            nc.sync.dma_start(out=h1cmp, in_=h1_dram[CS * c : CS * c + CS, :])
            wbt = wb_pool.tile([128, KT_BIL, 128], BF16, tag="wbt")
            if c < 2 or CFG.get("wb_memset_all"):
                # zero FWL pad cols once per pool buffer; later chunks reuse
                # the bytes and only rewrite [:, :, :CS]
                nc.vector.memset(wbt, 0.0)
            nc.sync.dma_start(out=wbt[:, :, :CS], in_=wbp[c])
            if not CFG["abl_repl"]:
                if "repl" not in _abl:
                    h0rep = []
                    for _ih in range(2):
                        hrt = rep_pool.tile([128, 5, BL], BF16, tag="h0rep")
                        nc.gpsimd.memset(hrt, 0.5)
                        h0rep.append(hrt)
                    h1dup = dup_pool.tile([128, 5, BL], BF16, tag="h1dup")
                    nc.gpsimd.memset(h1dup, 0.5)
                    _abl["repl"] = (h0rep, h1dup)
                h0rep, h1dup = _abl["repl"]
                return h0rep, h1dup, wbt, []
            h0rep = []
            for _ih in range(2):
                hrt = rep_pool.tile([128, 5, BL], BF16, tag="h0rep")
                h0rep.append(hrt)
            h1dup = dup_pool.tile([128, 5, BL], BF16, tag="h1dup")

            ops = []
            for h in range(NH):
                cols = slice(h * 512, (h + 1) * 512)
                for slot in range(NSLOT):
                    def op(h=h, cols=cols, slot=slot):
                        ps = rp_psum.tile([128, 512], F32, tag="rps")
                        if slot < 10:
                            src, dst = hev0, h0rep[slot // 5][:, slot % 5, cols]
                        else:
                            src, dst = h1cmp, h1dup[:, slot - 10, cols]
                        nc.tensor.matmul(
                            out=ps,
                            lhsT=lreps[:, slot, :],
                            rhs=src[:, cols],
                            start=True,
                            stop=True,
                            skip_group_check=True,
                        )
                        nc.scalar.copy(dst, ps)
                    ops.append(op)
            return h0rep, h1dup, wbt, ops

        zs_tiles = {}
        _abl = {}

        def emit_compute(c, feeds, bg):
            """Bilinear + post + norms for chunk c, draining bg ops (the
            next chunk's replica builds) between i-groups."""
            h0rep, h1dup, wbt = feeds
            bgi = 0

            av = post_pool.tile([CS, BL], BF16, tag="av")
            gv = post_pool.tile([CS, BL], BF16, tag="gv")
            nps = nsq_psum.tile([1, BL], F32, tag="nps")
            for h in range(NH):
                cols = slice(h * 512, (h + 1) * 512)
                if CFG["abl_bilmm"]:
                    zps = zpsum_pool.tile([128, 512], F32, tag="zps")
                else:
                    if "zps" not in _abl:
                        zps = zpsum_pool.tile([128, 512], F32, tag="zps")
                        nc.vector.memset(zps, 0.0)
                        _abl["zps"] = zps
                    zps = _abl["zps"]
                for i in range(10):
                    if CFG["abl_pt"]:
                        pt = p_pool.tile([128, 5, 512], BF16, tag="pt")
                        nc.vector.tensor_mul(
                            pt,
                            h0rep[i // 5][:, i % 5, cols]
                            .unsqueeze(1)
                            .broadcast_to([128, 5, 512]),
                            h1dup[:, :, cols],
                        )
                    else:
                        if "pt" not in _abl:
                            pts = []
                            for _pi in range(2):
                                pt = p_pool.tile([128, 5, 512], BF16, tag="pt")
                                nc.gpsimd.memset(pt, 0.5)
                                pts.append(pt)
                            _abl["pt"] = pts
                        pt = _abl["pt"][i % 2]
                    if CFG["abl_bilmm"]:
                        for j in range(5):
                            kt = 5 * i + j
                            nc.tensor.matmul(
                                out=zps,
                                lhsT=wbt[:, kt, :],
                                rhs=pt[:, j, :],
                                start=(kt == 0),
                                stop=(kt == KT_BIL - 1),
                            )
                    # drain up to 2 pending replica builds for chunk c+1
                    for _ in range(2):
                        if bgi < len(bg):
                            bg[bgi]()
                            bgi += 1
                nc.scalar.activation(
                    out=av[:, cols], in_=zps[:CS], func=AF.Abs,
                    bias=bbs[:, c : c + 1], scale=1.0,
                )
                nc.scalar.activation(
                    out=gv[:, cols], in_=zps[:CS], func=AF.Sign,
                    bias=bbs[:, c : c + 1], scale=1.0,
                )
                nc.tensor.matmul(
                    out=nps[:, cols],
                    lhsT=ones80,
                    rhs=av[:, cols],
                    start=True,
                    stop=True,
                    skip_group_check=True,
                )
            while bgi < len(bg):
                bg[bgi]()
                bgi += 1
            sv = post_pool.tile([CS, BL], BF16, tag="sv")
            nc.scalar.activation(out=sv, in_=av, func=AF.Sqrt)
            zst = zs_pool.tile([CS, BL], BF16, tag="zst")
            nc.gpsimd.tensor_mul(zst, sv, gv)
            zs_tiles[c] = zst
            nsq1 = nsq1_pool.tile([1, BL], F32, tag="nsq1")
            nc.scalar.copy(nsq1, nps)
            nc.sync.dma_start(out=nsq_big[c : c + 1, :], in_=nsq1)

            if c % HALF == HALF - 1:
                lo, hi = c - HALF + 1, c + 1
                # ACT/DVE need start-partition 0: recompute the whole strip
                nc.scalar.activation(out=rn_big, in_=nsq_big, func=AF.Sqrt)
                nc.vector.tensor_scalar_max(rn_big, rn_big, 1e-12)
                with nc.allow_low_precision(reason="rn applied to bf16 z"):
                    nc.vector.reciprocal(rn_bf16, rn_big)
                if not CFG["rnb_pe"]:
                    nc.sync.dma_start(
                        out=rn_dram[lo:hi, :], in_=rn_bf16[lo:hi]
                    )
                for cc in range(lo, hi):
                    # rn[cc] broadcast to 80 partitions via indicator matmul
                    rnb = rnb_pool.tile([CS, BL], BF16, tag="rnb")
                    if CFG["rnb_pe"]:
                        for h in range(NH):
                            cols = slice(h * 512, (h + 1) * 512)
                            ps = rp_psum.tile([128, 512], F32, tag="rps")
                            nc.tensor.matmul(
                                out=ps,
                                lhsT=lrns[:, cc, :],
                                rhs=rn_bf16[:, cols],
                                start=True,
                                stop=True,
                                skip_group_check=True,
                            )
                            nc.scalar.copy(rnb[:, cols], ps[:CS, :])
                    else:
                        nc.sync.dma_start(
                            out=rnb,
                            in_=rn_dram[cc : cc + 1, :].partition_broadcast(CS),
                        )
                    zn = zn_pool.tile([CS, BL], BF16, tag="zn")
                    nc.gpsimd.tensor_mul(zn, zs_tiles.pop(cc), rnb)
                    nc.sync.dma_start(
                        out=z_dram[CS * cc : CS * cc + CS, :], in_=zn
                    )

        feeds = {}
        prev = None
        for c in range(CHUNKS):
            hev0 = emit_proj(c, xt0, w0, b0s, h0e_pool, "h0ev")
            h0rep, h1dup, wbt, ops = make_replicas(c, hev0)
            feeds[c] = (h0rep, h1dup, wbt)
            if prev is None:
                for op in ops:
                    op()
            else:
                emit_compute(prev, feeds.pop(prev), ops)
            prev = c
        proj_es.close()
        out_pools = {
            "zk": ctx.enter_context(tc.tile_pool(name="zk", bufs=1)),
            "wo": ctx.enter_context(tc.tile_pool(name="wo", bufs=2)),
            "o": ctx.enter_context(tc.tile_pool(name="opool", bufs=2)),
        }
        emit_compute(prev, feeds.pop(prev), [])

        # ---- out projection: out^T = Wout^T @ z + bout ----
        ZKG = (5, 5, 3)  # k-tile groups for the z reload
        zk_pool = out_pools["zk"]
        wo_pool = out_pools["wo"]
        o_pool = out_pools["o"]
        zk = []
        kt0 = 0
        for gi, gn in enumerate(ZKG):
            zt = zk_pool.tile([128, gn, BL], BF16, tag=f"zk{gi}")
            nc.sync.dma_start(
                out=zt,
                in_=z_dram[kt0 * 128 : (kt0 + gn) * 128, :].rearrange(
                    "(kt p) b -> p kt b", p=128
                ),
            )
            for k in range(gn):
                zk.append((zt, k))
            kt0 += gn
        for mt in range(MT_O):
            m0 = mt * 128
            mw = min(128, D_OUT - m0)
            wot = wo_pool.tile([128, MT_H, 128], BF16, tag="wot")
            nc.sync.dma_start(
                out=wot.rearrange("p k m -> p (k m)"), in_=wout[:, mt, :]
            )
            ot = o_pool.tile(
                [128, BL], BF16 if CFG["out_bf16"] else F32, tag="ot"
            )
            for h in range(NH):
                ps = mm_psum.tile([128, 512], F32, tag="mmps")
                for kt in range(MT_H):
                    zt, k = zk[kt]
                    nc.tensor.matmul(
                        out=ps,
                        lhsT=wot[:, kt, :],
                        rhs=zt[:, k, h * 512 : (h + 1) * 512],
                        start=(kt == 0),
                        stop=(kt == MT_H - 1),
                    )
                nc.scalar.activation(
                    out=ot[:mw, h * 512 : (h + 1) * 512],
                    in_=ps[:mw, :],
                    func=AF.Identity,
                    bias=bouts[:mw, mt : mt + 1],
                    scale=1.0,
                )
            nc.sync.dma_start(out=outT[m0 : m0 + mw, :], in_=ot[:mw, :])


_PROGRAM = None


def _get_program():
    global _PROGRAM
    if _PROGRAM is None:
        _PROGRAM = build_program()
    return _PROGRAM


def prep_weights(W0, b0, W1, b1, Wb, bb, Wout, bout):
    bf = ml_dtypes.bfloat16

    def pack_proj_chunk(W):
        # [K, 1600] -> chunk-aligned [p, c, kt*128+m]: m-tile c holds the
        # chunk's 80 features in cols 0..79, cols 80..127 zero
        K = W.shape[0]
        Wp = np.zeros((K, CHUNKS * 128), np.float32)
        for c in range(CHUNKS):
            Wp[:, 128 * c : 128 * c + CS] = W[:, CS * c : CS * c + CS]
        kt_n = K // 128
        return np.ascontiguousarray(
            Wp.reshape(kt_n, 128, CHUNKS, 128).transpose(1, 2, 0, 3)
            .reshape(128, CHUNKS, kt_n * 128)
        ).astype(bf)

    def pack_bias_chunk(b):
        bp = np.zeros((128, CHUNKS), np.float32)
        for c in range(CHUNKS):
            bp[:CS, c] = b[CS * c : CS * c + CS]
        return np.ascontiguousarray(bp)

    w0 = pack_proj_chunk(np.asarray(W0, np.float32))
    w1 = pack_proj_chunk(np.asarray(W1, np.float32))
    b0cc = pack_bias_chunk(np.asarray(b0, np.float32))
    b1cc = pack_bias_chunk(np.asarray(b1, np.float32))
    # wbp[c, p, 5i+j, o] = Wb[c, o, 10*(p//16) + i, 5*(p%16) + j]
    p = np.arange(128)
    wbpt = np.empty((CHUNKS, 128, KT_BIL, CS), dtype=bf)
    for i in range(10):
        s_idx = 10 * (p // 16) + i
        for j in range(5):
            t_idx = 5 * (p % 16) + j
            wbpt[:, :, 5 * i + j, :] = Wb[:, :, s_idx, t_idx].transpose(0, 2, 1)
    # replica indicator matrices: slot<10: h0 row 10*(p//16)+i on partition
    # p; slot>=10: h1 row 5*(p%16)+j
    lrep = np.zeros((CS, NSLOT, 128), np.float32)
    for i in range(10):
        lrep[10 * (p // 16) + i, i, p] = 1.0
    for j in range(5):
        lrep[5 * (p % 16) + j, 10 + j, p] = 1.0
    lrepb = lrep.astype(bf)
    # rn broadcast indicators: out[p] = rn[cc]
    lrn = np.zeros((CHUNKS, CHUNKS, 128), np.float32)
    for cc in range(CHUNKS):
        lrn[cc, cc, :] = 1.0
    lrnb = lrn.astype(bf)
    Woutp = np.zeros((MM_PAD, D_OUT_PAD), np.float32)
    Woutp[:MM, :D_OUT] = Wout
    woutp = np.ascontiguousarray(
        Woutp.reshape(MT_H, 128, MT_O, 128).transpose(1, 2, 0, 3)
        .reshape(128, MT_O, MT_H * 128)
    ).astype(bf)
    bbT = np.ascontiguousarray(np.asarray(bb, np.float32).T)
    boutp = np.zeros(D_OUT_PAD, np.float32)
    boutp[:D_OUT] = bout
    boutc = np.ascontiguousarray(boutp.reshape(MT_O, 128).T)
    return dict(
        w0c=w0, w1c=w1, wbpt=wbpt, wout=woutp, b0cc=b0cc, b1cc=b1cc,
        bbT=bbT, boutc=boutc, lrep=lrepb, lrn=lrnb,
    )


def make_in_maps(x0, x1, weights):
    bf = ml_dtypes.bfloat16
    x0T = np.ascontiguousarray(np.asarray(x0, np.float32).T).astype(bf)
    x1T = np.ascontiguousarray(np.asarray(x1, np.float32).T).astype(bf)
    in_maps = []
    for r in range(N_CORES):
        sl = slice(r * BL, (r + 1) * BL)
        m = dict(weights)
        m["x0T"] = np.ascontiguousarray(x0T[:, sl])
        m["x1T"] = np.ascontiguousarray(x1T[:, sl])
        in_maps.append(m)
    return in_maps


def run(x0, x1, weights, **kwargs):
    nc = _get_program()
    in_maps = make_in_maps(x0, x1, weights)
    res = run_bass_kernel_spmd(nc, in_maps, core_ids=list(range(N_CORES)), **kwargs)
    out = np.empty((B, D_OUT), np.float32)
    for r in range(N_CORES):
        out[r * BL : (r + 1) * BL, :] = np.asarray(
            res.results[r]["outT"], np.float32
        ).T
    return out, res


def kernel(x0, x1, W0, b0, W1, b1, Wb, bb, Wout, bout):
    weights = prep_weights(W0, b0, W1, b1, Wb, bb, Wout, bout)
    out, _ = run(x0, x1, weights)
    return out


# ---- timed runner (no NTFF hook in this container: wall-clock the PJRT
# executable with device-resident inputs, minus dispatch overhead) ----

def _make_sharded_callable(nc, in_maps):
    import jax
    import numpy as _np
    from jax.sharding import Mesh, PartitionSpec, NamedSharding
    from jax.experimental.shard_map import shard_map
    from concourse import bass2jax as b2j
    from concourse import mybir as _mybir

    b2j.install_neuronx_cc_hook()
    n_cores = len(in_maps)
    partition_name = nc.partition_id_tensor.name if nc.partition_id_tensor else None
    in_names, out_names, out_avals, zero_outs = [], [], [], []
    for alloc in nc.m.functions[0].allocations:
        if not isinstance(alloc, _mybir.MemoryLocationSet):
            continue
        name = alloc.memorylocations[0].name
        if alloc.kind == "ExternalInput":
            if name != partition_name:
                in_names.append(name)
        elif alloc.kind == "ExternalOutput":
            shape = tuple(alloc.tensor_shape)
            dtype = _mybir.dt.np(alloc.dtype)
            out_names.append(name)
            out_avals.append(jax.core.ShapedArray(shape, dtype))
            zero_outs.append(_np.zeros(shape, dtype))
    n_params = len(in_names)
    in_names_all = list(in_names) + list(out_names)
    if partition_name is not None:
        in_names_all.append(partition_name)

    def _body(*args):
        operands = list(args)
        if partition_name is not None:
            operands.append(b2j.partition_id_tensor())
        outs = b2j._bass_exec_p.bind(
            *operands,
            out_avals=tuple(out_avals),
            in_names=tuple(in_names_all),
            out_names=tuple(out_names),
            lowering_input_output_aliases=(),
            sim_require_finite=True,
            sim_require_nnan=True,
            nc=nc,
        )
        return tuple(outs)

    devices = jax.devices()[:n_cores]
    mesh = Mesh(_np.asarray(devices), ("core",))
    spec = PartitionSpec("core")
    in_specs = (spec,) * (n_params + len(out_names))
    out_specs = (spec,) * len(out_names)
    n_outs = len(out_names)
    donate = tuple(range(n_params, n_params + n_outs))
    sharded = jax.jit(
        shard_map(_body, mesh=mesh, in_specs=in_specs, out_specs=out_specs,
                  check_rep=False),
        keep_unused=True,
        donate_argnums=donate,
    )
    sh = NamedSharding(mesh, spec)
    concat_in = [
        jax.device_put(
            _np.concatenate([_np.asarray(in_maps[c][n]) for c in range(n_cores)], 0), sh
        )
        for n in in_names
    ]
    state = {"outs": None}

    def _fresh_zeros():
        return [
            jax.device_put(_np.zeros((n_cores * z.shape[0], *z.shape[1:]), z.dtype), sh)
            for z in zero_outs
        ]

    def call():
        seeds = state["outs"] if state["outs"] is not None else _fresh_zeros()
        outs = sharded(*concat_in, *seeds)
        state["outs"] = list(outs)
        return outs
    return call, out_names, out_avals
